# revision 6
# baseline (speedup 1.0000x reference)
"""Trainium2 Bass kernel for the dense GNN message-passing step.

Computation (N=16384, NUM_IN=1024, NUM_OUT=256):
    states = zeros(N); states[input_indices] = input_values
    total  = states @ W + biases                      # GEMV over [N, N] f32
    out    = act_select(total)[output_indices]        # 0=id, 1=relu, 2=softsign

Strategy:
  * Only input_indices rows (1024) and output_indices columns (256) of W
    matter; the host gathers W[rows][:, cols] and shards 32 columns per
    core (tensor parallel per the sharding hint, applied post-gather).
  * ONE input DMA per core carries everything in a [128, 299] fp16
    payload: 11 "moving" columns (3 one-hot selectors + 8 x-chunks) and
    9 "stationary" [128, 32] chunks.  Chunk 0's first three partition
    rows hold per-column constants (bias, softsign-mask c2, relu floor
    L); chunks 1-8 are the weights.  Full 598 B contiguous rows keep the
    DMA at one packet per partition.
  * The GEMV accumulates in a [32-partition, 1] PSUM tile: stationary =
    weight chunk, moving = x chunk.  One-hot movings against chunk 0
    deliver bias into the accumulation and c2/L into their own PSUM
    tiles, so no memsets, no small DMAs, no scalar-engine ops.
  * Epilogue is 5 DVE ops using per-partition scalars straight out of
    PSUM:  a=|p|;  d=c2*a+1 (fused tensor_scalar);  r=recip(d);
    s=p*r;  out=max(s, L)  — L is 0 on relu columns, -1024 elsewhere,
    c2 is 1 on softsign columns.  relu/softsign/identity all emerge
    from the same arithmetic, no predication.
  * fp16 weights/x/bias with f32 PSUM accumulation lands ~2e-4 relative
    error vs the 2e-2 gate.
"""

import sys
import types

import numpy as np
from contextlib import ExitStack

import concourse.bacc as bacc
import concourse.tile as tile
from concourse import mybir
from concourse.bass_utils import run_bass_kernel_spmd


def _ensure_ntff_hook_module():
    """bass_utils imports antenv.axon_hooks when BASS_TRACE=1; some agent
    images ship antenv without that submodule, which would crash the run
    instead of degrading to trace-skip. Install a shim (backed by
    trn_agent_boot's ctypes NTFF driver when present) only if the real
    module is missing."""
    try:
        import antenv.axon_hooks  # noqa: F401
        return
    except ImportError:
        pass
    hook = [None]
    mod = types.ModuleType("antenv.axon_hooks")
    mod.set_axon_ntff_profile_hook = lambda h: hook.__setitem__(0, h)
    mod.get_axon_ntff_profile_hook = lambda: hook[0]
    try:
        import antenv as _antenv
        from trn_agent_boot.trn_boot import _ntff_profile_via_ctypes

        mod.set_axon_ntff_profile_hook(
            _ntff_profile_via_ctypes("/opt/axon/libaxon_pjrt.so")
        )
        sys.modules["antenv.axon_hooks"] = mod
        _antenv.axon_hooks = mod
    except Exception:
        sys.modules.setdefault("antenv.axon_hooks", mod)


_ensure_ntff_hook_module()

N_CORES = 8
K = 1024                 # padded contraction size (live rows)
KC = K // 128            # 8 k-chunks
NOUT = 256               # gathered output neurons
NPC = NOUT // N_CORES    # 32 output columns per core
NMOV = 3 + KC            # one-hot selectors e0,e1,e2 + 8 x chunks
NCH = 1 + KC             # const chunk + 8 weight chunks
C = NMOV + NCH * NPC     # 299 payload columns
L_NEG = -1024.0          # "no relu" floor (any value < -max|out| works)
F32 = mybir.dt.float32
F16 = mybir.dt.float16

_BUILT = None            # cached nc so repeat calls reuse the compiled module
LAST_RESULTS = None      # BassKernelResults of the most recent run (for test.py)


def _build_bass():
    nc = bacc.Bacc(
        "TRN2", target_bir_lowering=False, debug=False, num_devices=N_CORES
    )
    w = nc.dram_tensor("w", [128, C], F16, kind="ExternalInput").ap()
    o = nc.dram_tensor("o", [1, NPC], F32, kind="ExternalOutput").ap()

    with tile.TileContext(nc) as tc:
        with ExitStack() as ctx:
            wpool = ctx.enter_context(tc.tile_pool(name="wp", bufs=1))
            ppool = ctx.enter_context(tc.tile_pool(name="pp", bufs=1, space="PSUM"))
            scratch = ctx.enter_context(tc.tile_pool(name="scr", bufs=1))

            w_t = wpool.tile([128, C], F16, tag="w")
            nc.sync.dma_start(w_t[:], w[:])

            def mov(i):
                return w_t[:, i : i + 1]

            def chunk(c):
                return w_t[:, NMOV + c * NPC : NMOV + (c + 1) * NPC]

            p = ppool.tile([NPC, 1], F32, tag="p")
            pc2 = ppool.tile([NPC, 1], F32, tag="pc2")
            pl = ppool.tile([NPC, 1], F32, tag="pl")

            # chunk0 stationary: bias row -> p (opens accumulation),
            # c2 row -> pc2, L row -> pl.
            nc.tensor.matmul(p[:], chunk(0), mov(0), start=True, stop=False)
            nc.tensor.matmul(pc2[:], chunk(0), mov(1), start=True, stop=True)
            nc.tensor.matmul(pl[:], chunk(0), mov(2), start=True, stop=True)
            c2s = scratch.tile([NPC, 1], F32, tag="c2s")
            nc.vector.tensor_copy(c2s[:], pc2[:])    # runs during accumulation
            for c in range(1, NCH):
                nc.tensor.matmul(
                    p[:], chunk(c), mov(2 + c), start=False, stop=(c == NCH - 1)
                )

            # epilogue: softsign/relu/identity by arithmetic, all on DVE
            u = scratch.tile([NPC, 1], F32, tag="u")
            nc.vector.tensor_mul(u[:], p[:], c2s[:])  # c2*p
            a = scratch.tile([NPC, 1], F32, tag="a")
            nc.vector.scalar_tensor_tensor(          # |c2*p|
                a[:], u[:], -1.0, u[:],
                mybir.AluOpType.mult, mybir.AluOpType.max,
            )
            d = scratch.tile([NPC, 1], F32, tag="d")
            nc.vector.tensor_scalar_add(d[:], a[:], 1.0)
            r = scratch.tile([NPC, 1], F32, tag="r")
            nc.vector.reciprocal_approx_fast(out=r[:], in_=d[:])
            s = scratch.tile([NPC, 1], F32, tag="s")
            nc.vector.tensor_mul(s[:], p[:], r[:])   # p * r
            ot = scratch.tile([NPC, 1], F32, tag="ot")
            nc.vector.tensor_tensor(                 # max(s, L)
                ot[:], s[:], pl[:], mybir.AluOpType.max,
            )

            nc.scalar.dma_start(o[:], ot[:])

    nc.compile()
    return nc


def kernel(**inputs) -> np.ndarray:
    global _BUILT, LAST_RESULTS

    iv = np.asarray(inputs["input_values"], dtype=np.float32)
    W = np.asarray(inputs["weight_matrix"], dtype=np.float32)
    bias = np.asarray(inputs["biases"], dtype=np.float32)
    act = np.asarray(inputs["act_ids"])
    iidx = np.asarray(inputs["input_indices"]).astype(np.int64)
    oidx = np.asarray(inputs["output_indices"]).astype(np.int64)

    n = W.shape[0]
    # Dense neuron-state vector (duplicate indices: last write wins, matching
    # jax's .at[].set) and its index support.
    states = np.zeros(n, np.float32)
    states[iidx] = iv
    live = np.zeros(n, dtype=bool)
    live[iidx] = True
    support = np.flatnonzero(live)
    assert support.size <= K, "more than K live rows not supported"
    rows = np.zeros(K, np.int64)          # pad with row 0 (x=0 there => no-op)
    rows[: support.size] = support
    xvec = np.zeros(K, np.float32)
    xvec[: support.size] = states[support]

    in_maps = []
    for core in range(N_CORES):
        cols = oidx[core * NPC : (core + 1) * NPC]
        wh = np.zeros((128, C), np.float16)
        # one-hot movings
        wh[0, 0] = 1.0
        wh[1, 1] = 1.0
        wh[2, 2] = 1.0
        # x chunks: moving col 3+c, partition p = x[c*128+p]
        wh[:, 3 : 3 + KC] = xvec.reshape(KC, 128).T.astype(np.float16)
        # chunk 0: consts
        wh[0, NMOV : NMOV + NPC] = bias[cols].astype(np.float16)
        wh[1, NMOV : NMOV + NPC] = (act[cols] == 2).astype(np.float16)
        wh[2, NMOV : NMOV + NPC] = np.where(act[cols] == 1, 0.0, L_NEG).astype(
            np.float16
        )
        # chunks 1..8: weights, chunk c partition p = W[rows[(c-1)*128+p], col]
        ws = W[np.ix_(rows, cols)].astype(np.float16)     # [K, NPC]
        wh[:, NMOV + NPC :] = (
            ws.reshape(KC, 128, NPC).transpose(1, 0, 2).reshape(128, KC * NPC)
        )
        in_maps.append({"w": np.ascontiguousarray(wh)})

    if _BUILT is None:
        _BUILT = _build_bass()
    LAST_RESULTS = run_bass_kernel_spmd(
        _BUILT, in_maps, core_ids=list(range(N_CORES))
    )
    return np.concatenate(
        [LAST_RESULTS.results[c]["o"][0] for c in range(N_CORES)]
    ).astype(np.float32)


# revision 12
# speedup vs baseline: 1.2747x; 1.2747x over previous
"""Trainium2 Bass kernel for the dense GNN message-passing step.

Computation (N=16384, NUM_IN=1024, NUM_OUT=256):
    states = zeros(N); states[input_indices] = input_values
    total  = states @ W + biases                      # GEMV over [N, N] f32
    out    = act_select(total)[output_indices]        # 0=id, 1=relu, 2=softsign

Strategy:
  * Only input_indices rows (1024) and output_indices columns (256) of W
    matter; the host gathers W[rows][:, cols] and shards 32 columns per
    core (tensor parallel per the sharding hint, applied post-gather).
  * ONE input DMA per core carries everything in a [128, 299] fp16
    payload: 11 "moving" columns (3 one-hot selectors + 8 x-chunks) and
    9 [128, 32] weight chunks.  Chunk 0's first three partition rows
    hold per-column constants (bias, softsign-mask c2, relu floor L);
    chunks 1-8 are the weights.  Full 598 B contiguous rows keep the
    DMA at one packet per partition.
  * Raw bass (no TileContext), manual semaphores.  The kernel's
    instructions are spliced into the entry block right after each
    engine's preamble, BEFORE the framework's const-memset all-engine
    barrier — the input DMA and most of the compute overlap framework
    preamble time instead of following it.
  * GEMV accumulates into a [1, 32] PSUM tile: stationary = x-chunk
    column, moving = weight chunk.  One-hot movings against chunk 0
    deliver bias into the accumulation and c2/L into their own PSUM
    tiles: no memsets, no small DMAs, no scalar-engine ops.
  * Epilogue: 6 DVE ops, in-order on one engine (no inter-op sems):
    u=c2*p; a=|u|; d=a+1; r=recip(d); s=p*r; out=max(s, L).  L is 0 on
    relu columns, -1024 elsewhere; c2 is 1 on softsign columns — relu/
    softsign/identity all emerge from the same arithmetic.
  * fp16 weights/x/bias with f32 PSUM accumulation lands ~3e-4 relative
    error vs the 2e-2 gate.
"""

import sys
import types

import numpy as np

import concourse.bacc as bacc
from concourse import mybir
from concourse.bass_utils import run_bass_kernel_spmd


def _ensure_ntff_hook_module():
    """bass_utils imports antenv.axon_hooks when BASS_TRACE=1; some agent
    images ship antenv without that submodule, which would crash the run
    instead of degrading to trace-skip. Install a shim (backed by
    trn_agent_boot's ctypes NTFF driver when present) only if the real
    module is missing."""
    try:
        import antenv.axon_hooks  # noqa: F401
        return
    except ImportError:
        pass
    hook = [None]
    mod = types.ModuleType("antenv.axon_hooks")
    mod.set_axon_ntff_profile_hook = lambda h: hook.__setitem__(0, h)
    mod.get_axon_ntff_profile_hook = lambda: hook[0]
    try:
        import antenv as _antenv
        from trn_agent_boot.trn_boot import _ntff_profile_via_ctypes

        mod.set_axon_ntff_profile_hook(
            _ntff_profile_via_ctypes("/opt/axon/libaxon_pjrt.so")
        )
        sys.modules["antenv.axon_hooks"] = mod
        _antenv.axon_hooks = mod
    except Exception:
        sys.modules.setdefault("antenv.axon_hooks", mod)


_ensure_ntff_hook_module()

N_CORES = 8
K = 1024                 # padded contraction size (live rows)
KC = K // 128            # 8 k-chunks
NOUT = 256               # gathered output neurons
NPC = NOUT // N_CORES    # 32 output columns per core
NMOV = 3 + KC            # one-hot selectors e0,e1,e2 + 8 x chunks
NCH = 1 + KC             # const chunk + 8 weight chunks
C = NMOV + NCH * NPC     # 299 payload columns
L_NEG = -1024.0          # "no relu" floor (any value < -max|out| works)
F32 = mybir.dt.float32
F16 = mybir.dt.float16

_BUILT = None            # cached nc so repeat calls reuse the compiled module
LAST_RESULTS = None      # BassKernelResults of the most recent run (for test.py)


def _build_bass():
    nc = bacc.Bacc(
        "TRN2", target_bir_lowering=False, debug=False, num_devices=N_CORES
    )
    w = nc.dram_tensor("w", [128, C], F16, kind="ExternalInput").ap()
    o = nc.dram_tensor("o", [1, NPC], F32, kind="ExternalOutput").ap()

    w_t = nc.alloc_sbuf_tensor("w_t", [128, C], F16)
    c2s = nc.alloc_sbuf_tensor("c2s", [1, NPC], F32)
    u_t = nc.alloc_sbuf_tensor("u_t", [1, NPC], F32)
    a_t = nc.alloc_sbuf_tensor("a_t", [1, NPC], F32)
    d_t = nc.alloc_sbuf_tensor("d_t", [1, NPC], F32)
    r_t = nc.alloc_sbuf_tensor("r_t", [1, NPC], F32)
    s_t = nc.alloc_sbuf_tensor("s_t", [1, NPC], F32)
    o_t = nc.alloc_sbuf_tensor("o_t", [1, NPC], F32)
    p = nc.alloc_psum_tensor("p", [1, NPC], F32)
    pc2 = nc.alloc_psum_tensor("pc2", [1, NPC], F32)
    pl = nc.alloc_psum_tensor("pl", [1, NPC], F32)

    s_w = nc.alloc_semaphore("s_w")
    s_pc2 = nc.alloc_semaphore("s_pc2")
    s_p = nc.alloc_semaphore("s_p")
    s_v = nc.alloc_semaphore("s_v")      # DVE RAW-hazard chain (no HW interlock)
    s_epi = nc.alloc_semaphore("s_epi")
    s_out = nc.alloc_semaphore("s_out")

    mine = []

    def em(inst):
        mine.append(inst.ins)
        return inst

    wa = w_t.ap()

    def mov(i):
        return wa[:, i : i + 1]

    def chunk(c):
        return wa[:, NMOV + c * NPC : NMOV + (c + 1) * NPC]

    # --- SP: one big input DMA --------------------------------------- #
    em(nc.sync.dma_start(wa[:, :], w).then_inc(s_w, 16))

    # --- PE: const rows then the GEMV accumulation -------------------- #
    em(
        nc.tensor.matmul(pc2.ap(), mov(1), chunk(0), start=True, stop=True)
        ._wait_ge(s_w, 16)
        .then_inc(s_pc2, 1)
    )
    em(nc.tensor.matmul(pl.ap(), mov(2), chunk(0), start=True, stop=True))
    em(nc.tensor.matmul(p.ap(), mov(0), chunk(0), start=True, stop=False))
    for c in range(1, NCH):
        inst = nc.tensor.matmul(
            p.ap(), mov(2 + c), chunk(c), start=False, stop=(c == NCH - 1)
        )
        if c == NCH - 1:
            inst.then_inc(s_p, 1)
        em(inst)

    # --- DVE: epilogue.  The DVE pipelines back-to-back instructions
    # with no RAW interlock, so every dependent pair is chained through
    # s_v exactly like the tile framework does. ------------------------ #
    em(
        nc.vector.tensor_copy(c2s.ap(), pc2.ap())
        ._wait_ge(s_pc2, 1)
        .then_inc(s_v, 1)
    )
    em(
        nc.vector.tensor_mul(u_t.ap(), p.ap(), c2s.ap())     # c2*p
        ._wait_ge(s_p, 1)
        .wait_op(s_v, 1, "sem-ge", check=False)
        .then_inc(s_v, 1)
    )
    em(
        nc.vector.scalar_tensor_tensor(                      # |c2*p|
            a_t.ap(), u_t.ap(), -1.0, u_t.ap(),
            mybir.AluOpType.mult, mybir.AluOpType.max,
        )
        ._wait_ge(s_v, 2)
        .then_inc(s_v, 1)
    )
    em(
        nc.vector.tensor_scalar_add(d_t.ap(), a_t.ap(), 1.0)
        ._wait_ge(s_v, 3)
        .then_inc(s_v, 1)
    )
    em(
        nc.vector.reciprocal_approx_fast(out=r_t.ap(), in_=d_t.ap())
        ._wait_ge(s_v, 4)
        .then_inc(s_v, 1)
    )
    em(
        nc.vector.tensor_mul(s_t.ap(), p.ap(), r_t.ap())     # p * r
        ._wait_ge(s_v, 5)
        .then_inc(s_v, 1)
    )
    em(
        nc.vector.tensor_tensor(                             # max(s, L)
            o_t.ap(), s_t.ap(), pl.ap(), mybir.AluOpType.max,
        )
        ._wait_ge(s_v, 6)
        .then_inc(s_epi, 1)
    )

    # --- ACT: output DMA + completion fence --------------------------- #
    em(
        nc.scalar.dma_start(o, o_t.ap())
        ._wait_ge(s_epi, 1)
        .then_inc(s_out, 16)
    )
    em(nc.scalar.wait_ge(s_out, 16))

    # --- hoist: splice our instructions before the framework's
    # const-memset all-engine barrier, right after each engine's
    # preamble, so DMA + compute overlap the preamble dead zone. ------- #
    import os as _os
    _HOIST = _os.environ.get("KERNEL_HOIST", "1") == "1"
    blk = None
    for b in nc.main_func.blocks:
        ids = {id(i) for i in b.instructions}
        if id(mine[0]) in ids:
            blk = b
            break
    assert blk is not None, "could not locate kernel instructions"
    if _HOIST:
        myset = {id(i) for i in mine}
        blk.instructions[:] = [i for i in blk.instructions if id(i) not in myset]
        groups = {}
        for i in mine:
            groups.setdefault(i.engine, []).append(i)
        for eng in (nc.sync, nc.tensor, nc.vector, nc.scalar):
            lst = groups.pop(eng.engine, None)
            if not lst:
                continue
            pe = eng.preamble_end
            assert pe is not None
            at = blk.instructions.index(pe) + 1
            blk.instructions[at:at] = lst
        assert not groups, f"unplaced instruction groups: {list(groups)}"

    nc.compile()
    return nc


def kernel(**inputs) -> np.ndarray:
    global _BUILT, LAST_RESULTS

    iv = np.asarray(inputs["input_values"], dtype=np.float32)
    W = np.asarray(inputs["weight_matrix"], dtype=np.float32)
    bias = np.asarray(inputs["biases"], dtype=np.float32)
    act = np.asarray(inputs["act_ids"])
    iidx = np.asarray(inputs["input_indices"]).astype(np.int64)
    oidx = np.asarray(inputs["output_indices"]).astype(np.int64)

    n = W.shape[0]
    # Dense neuron-state vector (duplicate indices: last write wins, matching
    # jax's .at[].set) and its index support.
    states = np.zeros(n, np.float32)
    states[iidx] = iv
    live = np.zeros(n, dtype=bool)
    live[iidx] = True
    support = np.flatnonzero(live)
    assert support.size <= K, "more than K live rows not supported"
    rows = np.zeros(K, np.int64)          # pad with row 0 (x=0 there => no-op)
    rows[: support.size] = support
    xvec = np.zeros(K, np.float32)
    xvec[: support.size] = states[support]

    in_maps = []
    for core in range(N_CORES):
        cols = oidx[core * NPC : (core + 1) * NPC]
        wh = np.zeros((128, C), np.float16)
        # one-hot movings
        wh[0, 0] = 1.0
        wh[1, 1] = 1.0
        wh[2, 2] = 1.0
        # x chunks: moving col 3+c, partition p = x[c*128+p]
        wh[:, 3 : 3 + KC] = xvec.reshape(KC, 128).T.astype(np.float16)
        # chunk 0: consts
        wh[0, NMOV : NMOV + NPC] = bias[cols].astype(np.float16)
        wh[1, NMOV : NMOV + NPC] = (act[cols] == 2).astype(np.float16)
        wh[2, NMOV : NMOV + NPC] = np.where(act[cols] == 1, 0.0, L_NEG).astype(
            np.float16
        )
        # chunks 1..8: weights, chunk c partition p = W[rows[(c-1)*128+p], col]
        ws = W[np.ix_(rows, cols)].astype(np.float16)     # [K, NPC]
        wh[:, NMOV + NPC :] = (
            ws.reshape(KC, 128, NPC).transpose(1, 0, 2).reshape(128, KC * NPC)
        )
        in_maps.append({"w": np.ascontiguousarray(wh)})

    if _BUILT is None:
        _BUILT = _build_bass()
    LAST_RESULTS = run_bass_kernel_spmd(
        _BUILT, in_maps, core_ids=list(range(N_CORES))
    )
    return np.concatenate(
        [LAST_RESULTS.results[c]["o"][0] for c in range(N_CORES)]
    ).astype(np.float32)


# revision 14
# speedup vs baseline: 1.2842x; 1.0074x over previous
"""Trainium2 Bass kernel for the dense GNN message-passing step.

Computation (N=16384, NUM_IN=1024, NUM_OUT=256):
    states = zeros(N); states[input_indices] = input_values
    total  = states @ W + biases                      # GEMV over [N, N] f32
    out    = act_select(total)[output_indices]        # 0=id, 1=relu, 2=softsign

Strategy:
  * Only input_indices rows (1024) and output_indices columns (256) of W
    matter; the host gathers W[rows][:, cols] and shards 32 columns per
    core (tensor parallel per the sharding hint, applied post-gather).
  * ONE input DMA per core carries everything in a [128, 299] fp16
    payload: 11 "moving" columns (3 one-hot selectors + 8 x-chunks) and
    9 [128, 32] weight chunks.  Chunk 0's first three partition rows
    hold per-column constants (bias, softsign-mask c2, relu floor L);
    chunks 1-8 are the weights.  Full 598 B contiguous rows keep the
    DMA at one packet per partition.
  * Raw bass (no TileContext), manual semaphores.  The kernel's
    instructions are spliced into the entry block right after each
    engine's preamble, BEFORE the framework's const-memset all-engine
    barrier — the input DMA and most of the compute overlap framework
    preamble time instead of following it.
  * GEMV accumulates into a [1, 32] PSUM tile: stationary = x-chunk
    column, moving = weight chunk.  One-hot movings against chunk 0
    deliver bias into the accumulation and c2/L into their own PSUM
    tiles: no memsets, no small DMAs, no scalar-engine ops.
  * Epilogue: 6 DVE ops, in-order on one engine (no inter-op sems):
    u=c2*p; a=|u|; d=a+1; r=recip(d); s=p*r; out=max(s, L).  L is 0 on
    relu columns, -1024 elsewhere; c2 is 1 on softsign columns — relu/
    softsign/identity all emerge from the same arithmetic.
  * fp16 weights/x/bias with f32 PSUM accumulation lands ~3e-4 relative
    error vs the 2e-2 gate.
"""

import sys
import types

import numpy as np

import concourse.bacc as bacc
from concourse import mybir
from concourse.bass_utils import run_bass_kernel_spmd


def _ensure_ntff_hook_module():
    """bass_utils imports antenv.axon_hooks when BASS_TRACE=1; some agent
    images ship antenv without that submodule, which would crash the run
    instead of degrading to trace-skip. Install a shim (backed by
    trn_agent_boot's ctypes NTFF driver when present) only if the real
    module is missing."""
    try:
        import antenv.axon_hooks  # noqa: F401
        return
    except ImportError:
        pass
    hook = [None]
    mod = types.ModuleType("antenv.axon_hooks")
    mod.set_axon_ntff_profile_hook = lambda h: hook.__setitem__(0, h)
    mod.get_axon_ntff_profile_hook = lambda: hook[0]
    try:
        import antenv as _antenv
        from trn_agent_boot.trn_boot import _ntff_profile_via_ctypes

        mod.set_axon_ntff_profile_hook(
            _ntff_profile_via_ctypes("/opt/axon/libaxon_pjrt.so")
        )
        sys.modules["antenv.axon_hooks"] = mod
        _antenv.axon_hooks = mod
    except Exception:
        sys.modules.setdefault("antenv.axon_hooks", mod)


_ensure_ntff_hook_module()

N_CORES = 8
K = 1024                 # padded contraction size (live rows)
KC = K // 128            # 8 k-chunks
NOUT = 256               # gathered output neurons
NPC = NOUT // N_CORES    # 32 output columns per core
NMOV = 3 + KC            # one-hot selectors e0,e1,e2 + 8 x chunks
NCH = 1 + KC             # const chunk + 8 weight chunks
C = NMOV + NCH * NPC     # 299 payload columns
L_NEG = -1024.0          # "no relu" floor (any value < -max|out| works)
F32 = mybir.dt.float32
F16 = mybir.dt.float16

_BUILT = None            # cached nc so repeat calls reuse the compiled module
LAST_RESULTS = None      # BassKernelResults of the most recent run (for test.py)


def _build_bass():
    nc = bacc.Bacc(
        "TRN2", target_bir_lowering=False, debug=False, num_devices=N_CORES
    )
    w = nc.dram_tensor("w", [128, C], F16, kind="ExternalInput").ap()
    o = nc.dram_tensor("o", [1, NPC], F32, kind="ExternalOutput").ap()

    w_t = nc.alloc_sbuf_tensor("w_t", [128, C], F16)
    c2s = nc.alloc_sbuf_tensor("c2s", [1, NPC], F32)
    u_t = nc.alloc_sbuf_tensor("u_t", [1, NPC], F32)
    a_t = nc.alloc_sbuf_tensor("a_t", [1, NPC], F32)
    d_t = nc.alloc_sbuf_tensor("d_t", [1, NPC], F32)
    r_t = nc.alloc_sbuf_tensor("r_t", [1, NPC], F32)
    s_t = nc.alloc_sbuf_tensor("s_t", [1, NPC], F32)
    o_t = nc.alloc_sbuf_tensor("o_t", [1, NPC], F32)
    p = nc.alloc_psum_tensor("p", [1, NPC], F32)
    pc2 = nc.alloc_psum_tensor("pc2", [1, NPC], F32)
    pl = nc.alloc_psum_tensor("pl", [1, NPC], F32)

    s_w = nc.alloc_semaphore("s_w")
    s_pc2 = nc.alloc_semaphore("s_pc2")
    s_p = nc.alloc_semaphore("s_p")
    s_v = nc.alloc_semaphore("s_v")      # DVE RAW-hazard chain (no HW interlock)
    s_epi = nc.alloc_semaphore("s_epi")
    s_out = nc.alloc_semaphore("s_out")

    mine = []

    def em(inst):
        mine.append(inst.ins)
        return inst

    wa = w_t.ap()

    def mov(i):
        return wa[:, i : i + 1]

    def chunk(c):
        return wa[:, NMOV + c * NPC : NMOV + (c + 1) * NPC]

    # --- SP: one big input DMA --------------------------------------- #
    em(nc.sync.dma_start(wa[:, :], w).then_inc(s_w, 16))

    # --- PE: const rows then the GEMV accumulation -------------------- #
    em(
        nc.tensor.matmul(pc2.ap(), mov(1), chunk(0), start=True, stop=True)
        ._wait_ge(s_w, 16)
        .then_inc(s_pc2, 1)
    )
    em(nc.tensor.matmul(pl.ap(), mov(2), chunk(0), start=True, stop=True))
    em(nc.tensor.matmul(p.ap(), mov(0), chunk(0), start=True, stop=False))
    for c in range(1, NCH):
        inst = nc.tensor.matmul(
            p.ap(), mov(2 + c), chunk(c), start=False, stop=(c == NCH - 1)
        )
        if c == NCH - 1:
            inst.then_inc(s_p, 1)
        em(inst)

    # --- DVE: epilogue.  The DVE pipelines back-to-back instructions
    # with no RAW interlock, so every dependent pair is chained through
    # s_v exactly like the tile framework does. ------------------------ #
    em(
        nc.vector.tensor_copy(c2s.ap(), pc2.ap())
        ._wait_ge(s_pc2, 1)
        .then_inc(s_v, 1)
    )
    em(
        nc.vector.tensor_mul(u_t.ap(), p.ap(), c2s.ap())     # c2*p
        ._wait_ge(s_p, 1)
        .wait_op(s_v, 1, "sem-ge", check=False)
        .then_inc(s_v, 1)
    )
    em(
        nc.vector.scalar_tensor_tensor(                      # |c2*p|
            a_t.ap(), u_t.ap(), -1.0, u_t.ap(),
            mybir.AluOpType.mult, mybir.AluOpType.max,
        )
        ._wait_ge(s_v, 2)
        .then_inc(s_v, 1)
    )
    em(
        nc.vector.tensor_scalar_add(d_t.ap(), a_t.ap(), 1.0)
        ._wait_ge(s_v, 3)
        .then_inc(s_v, 1)
    )
    em(
        nc.vector.reciprocal_approx_fast(out=r_t.ap(), in_=d_t.ap())
        ._wait_ge(s_v, 4)
        .then_inc(s_v, 1)
    )
    em(
        nc.vector.tensor_mul(s_t.ap(), p.ap(), r_t.ap())     # p * r
        ._wait_ge(s_v, 5)
        .then_inc(s_v, 1)
    )
    em(
        nc.vector.tensor_tensor(                             # max(s, L)
            o_t.ap(), s_t.ap(), pl.ap(), mybir.AluOpType.max,
        )
        ._wait_ge(s_v, 6)
        .then_inc(s_epi, 1)
    )

    # --- ACT: output DMA + completion fence --------------------------- #
    em(
        nc.scalar.dma_start(o, o_t.ap())
        ._wait_ge(s_epi, 1)
        .then_inc(s_out, 16)
    )
    em(nc.scalar.wait_ge(s_out, 16))

    # --- hoist: splice our instructions before the framework's
    # const-memset all-engine barrier, right after each engine's
    # preamble, so DMA + compute overlap the preamble dead zone. ------- #
    import os as _os
    _HOIST = _os.environ.get("KERNEL_HOIST", "1")
    blk = None
    for b in nc.main_func.blocks:
        ids = {id(i) for i in b.instructions}
        if id(mine[0]) in ids:
            blk = b
            break
    assert blk is not None, "could not locate kernel instructions"
    if _HOIST in ("1", "pre"):
        myset = {id(i) for i in mine}
        anchors = {}
        if _HOIST == "pre":
            # anchor = first instruction of each engine (before its
            # register preamble) — executes before the wrapper's
            # ordering barrier and the window-opening const memsets.
            for i in blk.instructions:
                if id(i) in myset or type(i).__name__ == "InstCall":
                    continue
                anchors.setdefault(i.engine, i)
        blk.instructions[:] = [i for i in blk.instructions if id(i) not in myset]
        groups = {}
        for i in mine:
            groups.setdefault(i.engine, []).append(i)
        for eng in (nc.sync, nc.tensor, nc.vector, nc.scalar):
            lst = groups.pop(eng.engine, None)
            if not lst:
                continue
            if _HOIST == "pre" and eng.engine in anchors:
                at = blk.instructions.index(anchors[eng.engine])
            else:
                pe = eng.preamble_end
                assert pe is not None
                at = blk.instructions.index(pe) + 1
            blk.instructions[at:at] = lst
        assert not groups, f"unplaced instruction groups: {list(groups)}"

    nc.compile()
    return nc


def kernel(**inputs) -> np.ndarray:
    global _BUILT, LAST_RESULTS

    iv = np.asarray(inputs["input_values"], dtype=np.float32)
    W = np.asarray(inputs["weight_matrix"], dtype=np.float32)
    bias = np.asarray(inputs["biases"], dtype=np.float32)
    act = np.asarray(inputs["act_ids"])
    iidx = np.asarray(inputs["input_indices"]).astype(np.int64)
    oidx = np.asarray(inputs["output_indices"]).astype(np.int64)

    n = W.shape[0]
    # Dense neuron-state vector (duplicate indices: last write wins, matching
    # jax's .at[].set) and its index support.
    states = np.zeros(n, np.float32)
    states[iidx] = iv
    live = np.zeros(n, dtype=bool)
    live[iidx] = True
    support = np.flatnonzero(live)
    assert support.size <= K, "more than K live rows not supported"
    rows = np.zeros(K, np.int64)          # pad with row 0 (x=0 there => no-op)
    rows[: support.size] = support
    xvec = np.zeros(K, np.float32)
    xvec[: support.size] = states[support]

    in_maps = []
    for core in range(N_CORES):
        cols = oidx[core * NPC : (core + 1) * NPC]
        wh = np.zeros((128, C), np.float16)
        # one-hot movings
        wh[0, 0] = 1.0
        wh[1, 1] = 1.0
        wh[2, 2] = 1.0
        # x chunks: moving col 3+c, partition p = x[c*128+p]
        wh[:, 3 : 3 + KC] = xvec.reshape(KC, 128).T.astype(np.float16)
        # chunk 0: consts
        wh[0, NMOV : NMOV + NPC] = bias[cols].astype(np.float16)
        wh[1, NMOV : NMOV + NPC] = (act[cols] == 2).astype(np.float16)
        wh[2, NMOV : NMOV + NPC] = np.where(act[cols] == 1, 0.0, L_NEG).astype(
            np.float16
        )
        # chunks 1..8: weights, chunk c partition p = W[rows[(c-1)*128+p], col]
        ws = W[np.ix_(rows, cols)].astype(np.float16)     # [K, NPC]
        wh[:, NMOV + NPC :] = (
            ws.reshape(KC, 128, NPC).transpose(1, 0, 2).reshape(128, KC * NPC)
        )
        in_maps.append({"w": np.ascontiguousarray(wh)})

    if _BUILT is None:
        _BUILT = _build_bass()
    LAST_RESULTS = run_bass_kernel_spmd(
        _BUILT, in_maps, core_ids=list(range(N_CORES))
    )
    return np.concatenate(
        [LAST_RESULTS.results[c]["o"][0] for c in range(N_CORES)]
    ).astype(np.float32)
